# revision 1
# baseline (speedup 1.0000x reference)
"""Two-layer GCN (PyG GCNConv x2 + gelu + scaled residual) on 8 trn2 NeuronCores.

Strategy (per the sharding hint):
  - Nodes partitioned contiguously across the 8 cores (12500 each); edges
    assigned to the core owning their destination node.
  - 128x128 weights replicated; per-layer node-feature tables (xw = x @ W)
    are computed shard-wise and AllGathered so each core can gather the
    rows of its edges' source nodes ("halo exchange" of remote features).
  - Per dst-block (128 nodes) aggregation: gather y[src] rows with the
    custom SWDGE dma_gather, build a weighted one-hot selection matrix
    S_w[k, j] = norm[k] * (dst_local[k] == j) on the vector engine, and
    accumulate z += S_w.T @ G on the tensor engine in PSUM.
  - Degree/normalization (segment sums over static graph structure) and
    edge->slot layout are host-side preprocessing of the static graph.

Math:
  gcn(x, W, b) = dinv * (segsum_dst(w_e * y[src]) + y[i]) + b,
      where y = (x @ W) * dinv, dinv = rsqrt(deg + 1)
  equivalently with host-computed norm_e = dinv[src]*w*dinv[dst]:
      agg[i] = segsum_dst(norm_e * xw[src]) + dinv2[i]*xw[i] + b
  h   = gelu(agg1) + (x @ (0.3*Wres) + 0.3*bres)
  out = gelu(agg2(h))
"""

import numpy as np

P = 128
D = 128
NCORES = 8
NPC = 12500          # nodes per core
NBLK = 98            # 128-node blocks per core (98*128 = 12544)
NPCP = NBLK * P      # padded nodes per core
TR = NCORES * NPCP   # table rows (100352)
NRANGE = 4
RSZ = TR // NRANGE   # 25088 rows per gather range (< int16 max)
MAX_CHUNKS_PER_CALL = 8   # dma_gather num_idxs cap = 1024 = 8*128

_CACHE = {}


def _preprocess(x, edge_index, edge_weight, W1, b1, W2, b2, Wres, bres):
    BN = NCORES * NPC
    src = np.asarray(edge_index[0], dtype=np.int64)
    dst = np.asarray(edge_index[1], dtype=np.int64)
    w = np.asarray(edge_weight, dtype=np.float64)

    deg = np.bincount(dst, weights=w, minlength=BN) + 1.0
    dinv = 1.0 / np.sqrt(deg)
    norm_e = (dinv[src] * w * dinv[dst]).astype(np.float32)
    dinv2 = (dinv * dinv).astype(np.float32)

    trow_src = (src // NPC) * NPCP + (src % NPC)      # table row of src node
    core = dst // NPC
    loc = dst - core * NPC
    blk = loc // P
    dl = (loc % P).astype(np.float32)
    rng = trow_src // RSZ
    ridx = (trow_src % RSZ).astype(np.int16)

    # group edges by (core, blk, rng); stable order within groups
    order = np.lexsort((rng, blk, core))
    core_s, blk_s, rng_s = core[order], blk[order], rng[order]
    # per-edge position within its (core, blk, rng) group
    gid = (core_s * NBLK + blk_s) * NRANGE + rng_s
    ngroups = NCORES * NBLK * NRANGE
    cnt = np.bincount(gid, minlength=ngroups)
    start = np.concatenate([[0], np.cumsum(cnt)[:-1]])
    q = np.arange(len(gid)) - start[gid]

    # global chunk counts per (blk, rng): max over cores
    cnt3 = cnt.reshape(NCORES, NBLK, NRANGE)
    K = np.ceil(cnt3 / P).astype(np.int64).max(axis=0)  # [NBLK, NRANGE]
    K = np.maximum(K, 1)
    ktot = K.sum(axis=1)                                 # chunks per block
    cbase = np.zeros((NBLK, NRANGE), np.int64)           # chunk col base
    run = 0
    for b in range(NBLK):
        for r in range(NRANGE):
            cbase[b, r] = run
            run += K[b, r]
    C = int(run)                                         # total chunk cols

    # slot for each edge: chunk col cc, partition p
    cc = cbase[blk_s, rng_s] + q // P
    pp = q % P

    idx16 = np.zeros((NCORES, P, 8 * C), np.int16)
    dstl = np.zeros((NCORES, P, C), np.float32)
    normv = np.zeros((NCORES, P, C), np.float32)
    ic = 8 * cc + pp // 16
    ir = pp % 16
    idx16[core_s, ir, ic] = ridx[order]
    dstl[core_s, pp, cc] = dl[order]
    normv[core_s, pp, cc] = norm_e[order]
    for g in range(1, 8):
        idx16[:, 16 * g:16 * g + 16, :] = idx16[:, 0:16, :]

    dinv2o = np.zeros((NCORES, P, NBLK), np.float32)
    d2 = np.zeros(NCORES * NPCP, np.float32)
    for c in range(NCORES):
        d2[c * NPCP:c * NPCP + NPC] = dinv2[c * NPC:(c + 1) * NPC]
    dinv2o = d2.reshape(NCORES, NBLK, P).transpose(0, 2, 1).copy()

    xT = np.zeros((NCORES, P, NPCP), np.float32)
    xf = np.asarray(x, dtype=np.float32)
    for c in range(NCORES):
        xT[c, :, :NPC] = xf[c * NPC:(c + 1) * NPC].T

    iota = np.broadcast_to(np.arange(P, dtype=np.float32), (P, P)).copy()
    ident = np.eye(P, dtype=np.float32)

    consts = dict(
        W1=np.asarray(W1, np.float32), W2=np.asarray(W2, np.float32),
        Wres03=(0.3 * np.asarray(Wres, np.float32)),
        iota=iota, ident=ident,
    )
    b1 = np.asarray(b1, np.float32)
    b2 = np.asarray(b2, np.float32)
    bres03 = 0.3 * np.asarray(bres, np.float32)
    has_b1 = bool(np.any(b1)) or bool(np.any(bres03))
    has_b2 = bool(np.any(b2))
    if has_b1:
        # pre-gelu bias b1 broadcast; post-gelu bres03 folded into xr add
        consts["bias1"] = np.broadcast_to(b1, (P, P)).copy()
        consts["bres03"] = np.broadcast_to(bres03, (P, P)).copy()
    if has_b2:
        consts["bias2"] = np.broadcast_to(b2, (P, P)).copy()

    in_maps = []
    for c in range(NCORES):
        m = dict(consts)
        m.update(xT=xT[c], idx16=idx16[c], dstl=dstl[c], normv=normv[c],
                 dinv2o=dinv2o[c])
        in_maps.append(m)
    return K, has_b1, has_b2, in_maps


def _build(K, has_b1, has_b2):
    import concourse.bacc as bacc
    import concourse.bass as bass
    import concourse.mybir as mybir
    import concourse.tile as tile

    f32 = mybir.dt.float32
    C = int(K.sum())
    nc = bacc.Bacc(num_swdge_queues=4)

    xT_d = nc.dram_tensor("xT", [P, NPCP], f32, kind="ExternalInput")
    W1_d = nc.dram_tensor("W1", [P, P], f32, kind="ExternalInput")
    W2_d = nc.dram_tensor("W2", [P, P], f32, kind="ExternalInput")
    Wres_d = nc.dram_tensor("Wres03", [P, P], f32, kind="ExternalInput")
    iota_d = nc.dram_tensor("iota", [P, P], f32, kind="ExternalInput")
    ident_d = nc.dram_tensor("ident", [P, P], f32, kind="ExternalInput")
    idx_d = nc.dram_tensor("idx16", [P, 8 * C], mybir.dt.int16, kind="ExternalInput")
    dstl_d = nc.dram_tensor("dstl", [P, C], f32, kind="ExternalInput")
    norm_d = nc.dram_tensor("normv", [P, C], f32, kind="ExternalInput")
    dinv2_d = nc.dram_tensor("dinv2o", [P, NBLK], f32, kind="ExternalInput")
    bias1_d = nc.dram_tensor("bias1", [P, P], f32, kind="ExternalInput") if has_b1 else None
    bres_d = nc.dram_tensor("bres03", [P, P], f32, kind="ExternalInput") if has_b1 else None
    bias2_d = nc.dram_tensor("bias2", [P, P], f32, kind="ExternalInput") if has_b2 else None

    out_d = nc.dram_tensor("out", [NPCP, D], f32, kind="ExternalOutput")

    xw1_own = nc.dram_tensor("xw1_own", [NPCP, D], f32)
    xr03_dr = nc.dram_tensor("xr03", [NPCP, D], f32)
    xw2_own = nc.dram_tensor("xw2_own", [NPCP, D], f32)
    table1 = nc.dram_tensor("table1", [TR, D], f32, addr_space="Shared")
    table2 = nc.dram_tensor("table2", [TR, D], f32, addr_space="Shared")

    rg = [list(range(NCORES))]

    with tile.TileContext(nc) as tc:
        with (
            tc.tile_pool(name="meta", bufs=1) as mp,
            tc.tile_pool(name="gp", bufs=6) as gp,
            tc.tile_pool(name="wk", bufs=3) as wk,
            tc.tile_pool(name="pz", bufs=2, space="PSUM") as pz,
            tc.tile_pool(name="pa", bufs=2, space="PSUM") as pa,
        ):
            # ---- resident tiles
            w1_t = mp.tile([P, P], f32)
            w2_t = mp.tile([P, P], f32)
            wr_t = mp.tile([P, P], f32)
            iota_t = mp.tile([P, P], f32)
            id_t = mp.tile([P, P], f32)
            idx_t = mp.tile([P, 8 * C], mybir.dt.int16)
            dstl_t = mp.tile([P, C], f32)
            norm_t = mp.tile([P, C], f32)
            dinv2_t = mp.tile([P, NBLK], f32)
            hT_t = mp.tile([P, NPCP], f32)
            nc.sync.dma_start(out=w1_t[:], in_=W1_d[:])
            nc.sync.dma_start(out=w2_t[:], in_=W2_d[:])
            nc.sync.dma_start(out=wr_t[:], in_=Wres_d[:])
            nc.sync.dma_start(out=iota_t[:], in_=iota_d[:])
            nc.sync.dma_start(out=id_t[:], in_=ident_d[:])
            nc.sync.dma_start(out=idx_t[:], in_=idx_d[:])
            nc.sync.dma_start(out=dstl_t[:], in_=dstl_d[:])
            nc.sync.dma_start(out=norm_t[:], in_=norm_d[:])
            nc.sync.dma_start(out=dinv2_t[:], in_=dinv2_d[:])
            if has_b1:
                bias1_t = mp.tile([P, P], f32)
                bres_t = mp.tile([P, P], f32)
                nc.sync.dma_start(out=bias1_t[:], in_=bias1_d[:])
                nc.sync.dma_start(out=bres_t[:], in_=bres_d[:])
            if has_b2:
                bias2_t = mp.tile([P, P], f32)
                nc.sync.dma_start(out=bias2_t[:], in_=bias2_d[:])

            # ---- phase A: xw1 = x@W1, xr03 = x@(0.3*Wres), shard-local
            for t in range(NBLK):
                xt = wk.tile([P, P], f32, tag="xt")
                nc.sync.dma_start(out=xt[:], in_=xT_d[:, t * P:(t + 1) * P])
                ps1 = pa.tile([P, P], f32, space="PSUM", tag="ps1")
                ps2 = pa.tile([P, P], f32, space="PSUM", tag="ps2")
                nc.tensor.matmul(ps1[:], xt[:], w1_t[:], start=True, stop=True)
                nc.tensor.matmul(ps2[:], xt[:], wr_t[:], start=True, stop=True)
                c1 = wk.tile([P, P], f32, tag="c1")
                c2 = wk.tile([P, P], f32, tag="c2")
                nc.vector.tensor_copy(out=c1[:], in_=ps1[:])
                if has_b1:
                    nc.vector.tensor_add(out=c2[:], in0=ps2[:], in1=bres_t[:])
                else:
                    nc.vector.tensor_copy(out=c2[:], in_=ps2[:])
                nc.sync.dma_start(out=xw1_own[t * P:(t + 1) * P, :], in_=c1[:])
                nc.sync.dma_start(out=xr03_dr[t * P:(t + 1) * P, :], in_=c2[:])

            nc.gpsimd.collective_compute(
                "AllGather", mybir.AluOpType.bypass, replica_groups=rg,
                ins=[xw1_own[:]], outs=[table1[:]],
            )

            # ---- per-layer edge aggregation pass
            def layer_pass(table_d, own_d, layer):
                cc = 0
                for b in range(NBLK):
                    zp = pz.tile([P, P], f32, space="PSUM", tag="z")
                    nch = int(K[b].sum())
                    ci = 0
                    for r in range(NRANGE):
                        kc = int(K[b, r])
                        j0 = 0
                        while j0 < kc:
                            ncall = min(MAX_CHUNKS_PER_CALL, kc - j0)
                            gb = gp.tile([P, ncall, D], f32, tag="g")
                            col0 = cc + j0
                            nc.gpsimd.dma_gather(
                                out_ap=gb[:],
                                in_ap=table_d[r * RSZ:(r + 1) * RSZ, :],
                                idxs_ap=idx_t[:, 8 * col0:8 * (col0 + ncall)],
                                num_idxs=P * ncall,
                                num_idxs_reg=P * ncall,
                                elem_size=D,
                                queue_num=r % 4,
                            )
                            for j in range(ncall):
                                col = col0 + j
                                sw = wk.tile([P, P], f32, tag="sw")
                                nc.vector.tensor_scalar(
                                    out=sw[:], in0=iota_t[:],
                                    scalar1=dstl_t[:, col:col + 1],
                                    scalar2=norm_t[:, col:col + 1],
                                    op0=mybir.AluOpType.is_equal,
                                    op1=mybir.AluOpType.mult,
                                )
                                nc.tensor.matmul(
                                    zp[:], sw[:], gb[:, j, :],
                                    start=(ci == 0), stop=(ci == nch - 1),
                                )
                                ci += 1
                            j0 += ncall
                        cc += kc
                    # epilogue
                    ob = wk.tile([P, P], f32, tag="ob")
                    nc.sync.dma_start(out=ob[:], in_=own_d[b * P:(b + 1) * P, :])
                    e1 = wk.tile([P, P], f32, tag="e1")
                    nc.vector.tensor_scalar(
                        out=e1[:], in0=ob[:],
                        scalar1=dinv2_t[:, b:b + 1], scalar2=None,
                        op0=mybir.AluOpType.mult,
                    )
                    e2 = wk.tile([P, P], f32, tag="e2")
                    nc.vector.tensor_add(out=e2[:], in0=zp[:], in1=e1[:])
                    if layer == 1 and has_b1:
                        nc.vector.tensor_add(out=e2[:], in0=e2[:], in1=bias1_t[:])
                    if layer == 2 and has_b2:
                        nc.vector.tensor_add(out=e2[:], in0=e2[:], in1=bias2_t[:])
                    ge = wk.tile([P, P], f32, tag="ge")
                    nc.scalar.activation(
                        out=ge[:], in_=e2[:],
                        func=mybir.ActivationFunctionType.Gelu,
                    )
                    if layer == 1:
                        xr = wk.tile([P, P], f32, tag="xr")
                        nc.sync.dma_start(out=xr[:], in_=xr03_dr[b * P:(b + 1) * P, :])
                        hb = wk.tile([P, P], f32, tag="hb")
                        nc.vector.tensor_add(out=hb[:], in0=ge[:], in1=xr[:])
                        pt = pz.tile([P, P], f32, space="PSUM", tag="pt")
                        nc.tensor.transpose(out=pt[:], in_=hb[:], identity=id_t[:])
                        nc.vector.tensor_copy(out=hT_t[:, b * P:(b + 1) * P], in_=pt[:])
                    else:
                        nc.sync.dma_start(out=out_d[b * P:(b + 1) * P, :], in_=ge[:])

            layer_pass(table1, xw1_own, 1)

            # ---- phase C: xw2 = h @ W2 (from SBUF-resident hT)
            for t in range(NBLK):
                psC = pa.tile([P, P], f32, space="PSUM", tag="ps1")
                nc.tensor.matmul(psC[:], hT_t[:, t * P:(t + 1) * P], w2_t[:],
                                 start=True, stop=True)
                cC = wk.tile([P, P], f32, tag="c1")
                nc.vector.tensor_copy(out=cC[:], in_=psC[:])
                nc.sync.dma_start(out=xw2_own[t * P:(t + 1) * P, :], in_=cC[:])

            nc.gpsimd.collective_compute(
                "AllGather", mybir.AluOpType.bypass, replica_groups=rg,
                ins=[xw2_own[:]], outs=[table2[:]],
            )

            layer_pass(table2, xw2_own, 2)

    nc.compile()
    return nc


def _get_compiled(K, has_b1, has_b2):
    key = (K.tobytes(), has_b1, has_b2)
    if key not in _CACHE:
        _CACHE[key] = _build(K, has_b1, has_b2)
    return _CACHE[key]


def kernel(x, edge_index, B, N, causal_edge_index, edge_weight,
           causal_edge_weight, W1, b1, W2, b2, Wres, bres):
    assert int(B) * int(N) == NCORES * NPC
    from concourse.bass_utils import run_bass_kernel_spmd

    K, has_b1, has_b2, in_maps = _preprocess(
        x, edge_index, edge_weight, W1, b1, W2, b2, Wres, bres)
    nc = _get_compiled(K, has_b1, has_b2)
    res = run_bass_kernel_spmd(nc, in_maps, list(range(NCORES)))
    out = np.concatenate(
        [res.results[c]["out"][:NPC] for c in range(NCORES)], axis=0)
    return out.astype(np.float32)


# exposed for test.py so it can reuse preprocessing + run with tracing
def _run_traced(x, edge_index, edge_weight, W1, b1, W2, b2, Wres, bres,
                **trace_kwargs):
    from concourse.bass_utils import run_bass_kernel_spmd
    K, has_b1, has_b2, in_maps = _preprocess(
        x, edge_index, edge_weight, W1, b1, W2, b2, Wres, bres)
    nc = _get_compiled(K, has_b1, has_b2)
    res = run_bass_kernel_spmd(nc, in_maps, list(range(NCORES)),
                               **trace_kwargs)
    out = np.concatenate(
        [res.results[c]["out"][:NPC] for c in range(NCORES)], axis=0)
    return out.astype(np.float32), res



# revision 13
# speedup vs baseline: 2.2521x; 2.2521x over previous
"""Two-layer GCN (PyG GCNConv x2 + gelu + scaled residual) on 8 trn2 NeuronCores.

Strategy (per the sharding hint):
  - Nodes partitioned contiguously across the 8 cores (12500 each); edges
    assigned to the core owning their destination node.
  - 128x128 weights replicated; per-layer node-feature tables (xw = x @ W)
    are computed shard-wise and AllGathered (in 4 pipelined quarter
    collectives) so each core can gather the rows of its edges' source
    nodes ("halo exchange").
  - Aggregation: per dst-block (128 nodes), gather y[src] rows in fp16 via
    SWDGE dma_gather (4 queues round-robin = 4 Q7 core pairs), build a
    weighted one-hot selection S_w[k, j] = norm[k] * (dst_local[k] == j)
    with batched broadcast-AP tensor_tensor ops, and accumulate
    z += S_w.T @ G on the tensor engine (fp16 in, fp32 PSUM accumulate).
  - Range-major processing with an SBUF fp16 accumulator so each table
    quarter is consumed as soon as its collective lands.
  - Degree/normalization and edge->slot layout are host-side preprocessing
    of the static graph structure. Trailing padding slots use idx=-1 which
    the gather ucode truncates (no descriptor cost).

Math:
  gcn(x, W, b) = dinv * (segsum_dst(w_e * y[src]) + y[i]) + b,
      where y = (x @ W) * dinv, dinv = rsqrt(deg + 1)
  equivalently with host-computed norm_e = dinv[src]*w*dinv[dst]:
      agg[i] = segsum_dst(norm_e * xw[src]) + dinv2[i]*xw[i] + b
  h   = gelu(agg1) + (x @ (0.3*Wres) + 0.3*bres)
  out = gelu(agg2(h))
"""

import os

import numpy as np

V_AGINLINE = os.environ.get("V_AGINLINE", "1") == "1"   # quarter-AGs inline vs at phase end
V_REG = os.environ.get("V_REG", "1") == "1"             # per-core num_idxs_reg + trailing -1
V_BATCH = os.environ.get("V_BATCH", "1") == "1"         # batched broadcast-AP S_w builds
V_STAGE = os.environ.get("V_STAGE", "FULL")             # A | A1 | A1E | FULL

P = 128
D = 128
NCORES = 8
NPC = 12500          # nodes per core
NBLK = 98            # 128-node blocks per core (98*128 = 12544)
NPCP = NBLK * P      # padded nodes per core
NRANGE = 4
QB = [25, 25, 24, 24]            # blocks per quarter (sum = 98)
QSTART = [0, 25, 50, 74]         # first block of each quarter
QROWS = [q * P for q in QB]      # rows per quarter per core
RS = [NCORES * qr for qr in QROWS]   # table rows per range
RBASE = [0, RS[0], RS[0] + RS[1], RS[0] + RS[1] + RS[2]]
TR = sum(RS)         # table rows (100352)
MAXCH = 8            # max chunks per dma_gather call (1024 idxs; ring cap)

_CACHE = {}


def _preprocess(x, edge_index, edge_weight, W1, b1, W2, b2, Wres, bres):
    BN = NCORES * NPC
    src = np.asarray(edge_index[0], dtype=np.int64)
    dst = np.asarray(edge_index[1], dtype=np.int64)
    w = np.asarray(edge_weight, dtype=np.float64)

    deg = np.bincount(dst, weights=w, minlength=BN) + 1.0
    dinv = 1.0 / np.sqrt(deg)
    norm_e = (dinv[src] * w * dinv[dst]).astype(np.float32)
    dinv2 = (dinv * dinv).astype(np.float32)

    # --- source node -> (range, ridx) via quarter-major table layout
    s_core = src // NPC
    s_loc = src - s_core * NPC
    s_blk = s_loc // P
    qof = np.zeros(NBLK, np.int64)
    for q in range(NRANGE):
        qof[QSTART[q]:QSTART[q] + QB[q]] = q
    rng = qof[s_blk]
    qstart_rows = np.array([QSTART[q] * P for q in range(NRANGE)])
    qrows = np.array(QROWS)
    ridx = (s_core * qrows[rng] + (s_loc - qstart_rows[rng])).astype(np.int16)

    # --- destination -> (core, block, dst-local)
    core = dst // NPC
    loc = dst - core * NPC
    blk = loc // P
    dl = (loc % P).astype(np.float32)

    # --- group edges by (core, rng, blk); stable order within groups
    order = np.lexsort((blk, rng, core))
    core_s, blk_s, rng_s = core[order], blk[order], rng[order]
    gid = (core_s * NRANGE + rng_s) * NBLK + blk_s
    ngroups = NCORES * NRANGE * NBLK
    cnt = np.bincount(gid, minlength=ngroups)
    start = np.concatenate([[0], np.cumsum(cnt)[:-1]])
    q = np.arange(len(gid)) - start[gid]

    # static chunk structure: K[r, b] = max over cores
    cnt3 = cnt.reshape(NCORES, NRANGE, NBLK)
    K = np.maximum(np.ceil(cnt3 / P).astype(np.int64).max(axis=0), 1)  # [NRANGE, NBLK]

    # chunk column bases, range-major then block
    cbase = np.zeros((NRANGE, NBLK), np.int64)
    run = 0
    for r in range(NRANGE):
        for b in range(NBLK):
            cbase[r, b] = run
            run += K[r, b]
    C = int(run)

    # gather call packing: per range, greedy consecutive groups, <= MAXCH chunks
    # call = (r, c0, ncall, groups=[(b, gcol0, kk), ...])
    calls = []
    for r in range(NRANGE):
        b = 0
        while b < NBLK:
            c0 = int(cbase[r, b])
            ncall = 0
            groups = []
            while b < NBLK and ncall + int(K[r, b]) <= MAXCH:
                groups.append((b, int(cbase[r, b]), int(K[r, b])))
                ncall += int(K[r, b])
                b += 1
            calls.append((r, c0, ncall, groups))

    # per-edge slot: chunk col cc, partition pp
    cc = cbase[rng_s, blk_s] + q // P
    pp = q % P
    flat = cc * P + pp  # flat slot id in [0, C*P)

    slot_idx = np.zeros((NCORES, C * P), np.int16)
    slot_dstl = np.zeros((NCORES, C * P), np.float32)
    slot_norm = np.zeros((NCORES, C * P), np.float32)
    slot_idx[core_s, flat] = ridx[order]
    slot_dstl[core_s, flat] = dl[order]
    slot_norm[core_s, flat] = norm_e[order]

    # Per-(core, call) runtime gather count: whole trailing padding chunks are
    # marked -1 (the gather ucode truncates trailing negatives) and the count
    # register is rounded to the same 128 boundary so the decode-side ring
    # reservation matches the ucode's descriptor count exactly. Partial-chunk
    # padding keeps idx 0 (dummy descriptors cost the same either way).
    real = np.zeros((NCORES, C * P), bool)
    real[core_s, flat] = True
    nidx_call = np.zeros((NCORES, len(calls)), np.int32)
    for ci, (_r, c0, ncall, _g) in enumerate(calls):
        s0, s1 = c0 * P, (c0 + ncall) * P
        seg = real[:, s0:s1]
        rev = seg[:, ::-1]
        has = rev.any(axis=1)
        last = np.where(has, (s1 - s0 - 1) - rev.argmax(axis=1), -1)
        kept = (last // P + 1) * P          # 128-rounded per-core count
        nidx_call[:, ci] = kept
        if V_REG:
            mask = np.arange(s1 - s0)[None, :] >= kept[:, None]
            slot_idx[:, s0:s1][mask] = -1

    # idx16 layout [16, 8*C] wrapped, replicated to 128 partitions
    arr = slot_idx.reshape(NCORES, C, 8, 16)            # [c, cc, pp//16, pp%16]
    idx16s = arr.transpose(0, 3, 1, 2).reshape(NCORES, 16, 8 * C)
    idx16 = np.tile(idx16s, (1, 8, 1))                  # [c, 128, 8C]

    dstl = slot_dstl.reshape(NCORES, C, P).transpose(0, 2, 1).astype(np.float16)
    normv = slot_norm.reshape(NCORES, C, P).transpose(0, 2, 1).astype(np.float16)

    d2 = np.zeros((NCORES, NPCP), np.float32)
    for c in range(NCORES):
        d2[c, :NPC] = dinv2[c * NPC:(c + 1) * NPC]
    dinv2o = d2.reshape(NCORES, NBLK, P).transpose(0, 2, 1).copy()

    xT = np.zeros((NCORES, P, NPCP), np.float16)
    xf = np.asarray(x, dtype=np.float32)
    for c in range(NCORES):
        xT[c, :, :NPC] = xf[c * NPC:(c + 1) * NPC].T.astype(np.float16)

    iota = np.broadcast_to(np.arange(P, dtype=np.float16), (P, P)).copy()
    ident = np.eye(P, dtype=np.float32)

    consts = dict(
        W1=np.asarray(W1, np.float32).astype(np.float16),
        W2=np.asarray(W2, np.float32).astype(np.float16),
        Wres03=(0.3 * np.asarray(Wres, np.float32)).astype(np.float16),
        iota=iota, ident=ident,
    )
    b1 = np.asarray(b1, np.float32)
    b2 = np.asarray(b2, np.float32)
    bres03 = 0.3 * np.asarray(bres, np.float32)
    has_b1 = bool(np.any(b1)) or bool(np.any(bres03))
    has_b2 = bool(np.any(b2))
    if has_b1:
        consts["bias1"] = np.broadcast_to(b1, (P, P)).copy()
        consts["bres03"] = np.broadcast_to(bres03, (P, P)).copy()
    if has_b2:
        consts["bias2"] = np.broadcast_to(b2, (P, P)).copy()

    in_maps = []
    for c in range(NCORES):
        m = dict(consts)
        m.update(xT=xT[c], idx16=idx16[c], dstl=dstl[c], normv=normv[c],
                 dinv2o=dinv2o[c], ncnt=nidx_call[c:c + 1])
        in_maps.append(m)
    call_sig = tuple((r, c0, ncall, tuple(g)) for r, c0, ncall, g in calls)
    return C, call_sig, has_b1, has_b2, in_maps


def _build(C, calls, has_b1, has_b2):
    import concourse.bacc as bacc
    import concourse.bass as bass
    import concourse.mybir as mybir
    import concourse.tile as tile

    f32 = mybir.dt.float32
    f16 = mybir.dt.float16
    i16 = mybir.dt.int16
    EQ = mybir.AluOpType.is_equal
    MUL = mybir.AluOpType.mult
    ADD = mybir.AluOpType.add
    GELU = mybir.ActivationFunctionType.Gelu
    COPY = mybir.ActivationFunctionType.Copy

    nc = bacc.Bacc(num_swdge_queues=4)

    xT_d = nc.dram_tensor("xT", [P, NPCP], f16, kind="ExternalInput")
    W1_d = nc.dram_tensor("W1", [P, P], f16, kind="ExternalInput")
    W2_d = nc.dram_tensor("W2", [P, P], f16, kind="ExternalInput")
    Wres_d = nc.dram_tensor("Wres03", [P, P], f16, kind="ExternalInput")
    iota_d = nc.dram_tensor("iota", [P, P], f16, kind="ExternalInput")
    ident_d = nc.dram_tensor("ident", [P, P], f32, kind="ExternalInput")
    idx_d = nc.dram_tensor("idx16", [P, 8 * C], i16, kind="ExternalInput")
    dstl_d = nc.dram_tensor("dstl", [P, C], f16, kind="ExternalInput")
    norm_d = nc.dram_tensor("normv", [P, C], f16, kind="ExternalInput")
    dinv2_d = nc.dram_tensor("dinv2o", [P, NBLK], f32, kind="ExternalInput")
    ncnt_d = nc.dram_tensor("ncnt", [1, len(calls)], mybir.dt.int32,
                            kind="ExternalInput")
    bias1_d = nc.dram_tensor("bias1", [P, P], f32, kind="ExternalInput") if has_b1 else None
    bres_d = nc.dram_tensor("bres03", [P, P], f32, kind="ExternalInput") if has_b1 else None
    bias2_d = nc.dram_tensor("bias2", [P, P], f32, kind="ExternalInput") if has_b2 else None

    out_d = nc.dram_tensor("out", [NPCP, D], f32, kind="ExternalOutput")

    xw1_own = nc.dram_tensor("xw1_own", [NPCP, D], f16)
    xr03_dr = nc.dram_tensor("xr03", [NPCP, D], f16)
    xw2_own = nc.dram_tensor("xw2_own", [NPCP, D], f16)
    table1 = nc.dram_tensor("table1", [TR, D], f16, addr_space="Shared")
    table2 = nc.dram_tensor("table2", [TR, D], f16, addr_space="Shared")

    rg = [list(range(NCORES))]
    GBUFS = 6
    qctr = [0]

    with tile.TileContext(nc) as tc:
        with (
            tc.tile_pool(name="meta", bufs=1) as mp,
            tc.tile_pool(name="gp", bufs=GBUFS) as gp,
            tc.tile_pool(name="sp", bufs=4) as sp,
            tc.tile_pool(name="wk", bufs=3) as wk,
            tc.tile_pool(name="pz", bufs=3, space="PSUM") as pz,
            tc.tile_pool(name="pa", bufs=2, space="PSUM") as pa,
        ):
            # ---- resident tiles
            w1_t = mp.tile([P, P], f16)
            w2_t = mp.tile([P, P], f16)
            wr_t = mp.tile([P, P], f16)
            iota_t = mp.tile([P, P], f16)
            id_t = mp.tile([P, P], f32)
            idx_t = mp.tile([P, 8 * C], i16)
            dstl_t = mp.tile([P, C], f16)
            norm_t = mp.tile([P, C], f16)
            dinv2_t = mp.tile([P, NBLK], f32)
            zacc = mp.tile([P, NBLK * P], f16)
            ncnt_t = mp.tile([1, len(calls)], mybir.dt.int32)
            nc.sync.dma_start(out=ncnt_t[:], in_=ncnt_d[:])
            nc.sync.dma_start(out=w1_t[:], in_=W1_d[:])
            nc.sync.dma_start(out=w2_t[:], in_=W2_d[:])
            nc.sync.dma_start(out=wr_t[:], in_=Wres_d[:])
            nc.sync.dma_start(out=iota_t[:], in_=iota_d[:])
            nc.sync.dma_start(out=id_t[:], in_=ident_d[:])
            nc.sync.dma_start(out=idx_t[:], in_=idx_d[:])
            nc.sync.dma_start(out=dstl_t[:], in_=dstl_d[:])
            nc.sync.dma_start(out=norm_t[:], in_=norm_d[:])
            nc.sync.dma_start(out=dinv2_t[:], in_=dinv2_d[:])
            if has_b1:
                bias1_t = mp.tile([P, P], f32)
                bres_t = mp.tile([P, P], f32)
                nc.sync.dma_start(out=bias1_t[:], in_=bias1_d[:])
                nc.sync.dma_start(out=bres_t[:], in_=bres_d[:])
            if has_b2:
                bias2_t = mp.tile([P, P], f32)
                nc.sync.dma_start(out=bias2_t[:], in_=bias2_d[:])

            # zero the gather buffers once: truncated gather slots leave SBUF
            # untouched, and the matmul multiplies them by 0 — stale NaNs from
            # uninitialized SBUF would poison 0*NaN. After this, reuses only
            # ever contain previous gathered table rows (finite).
            for _ in range(GBUFS):
                g0 = gp.tile([P, MAXCH, D], f16, tag="g")
                nc.vector.memset(g0[:], 0)

            # ---- phase A: xw1 = x@W1, xr03 = x@(0.3*Wres), shard-local;
            # fire quarter-AllGathers as their blocks complete.
            qend = [QSTART[j] + QB[j] - 1 for j in range(NRANGE)]
            for t in range(NBLK):
                xt = wk.tile([P, P], f16, tag="xt")
                nc.sync.dma_start(out=xt[:], in_=xT_d[:, t * P:(t + 1) * P])
                ps1 = pa.tile([P, P], f32, space="PSUM", tag="ps1")
                ps2 = pa.tile([P, P], f32, space="PSUM", tag="ps2")
                nc.tensor.matmul(ps1[:], xt[:], w1_t[:], start=True, stop=True)
                nc.tensor.matmul(ps2[:], xt[:], wr_t[:], start=True, stop=True)
                c1 = wk.tile([P, P], f16, tag="c1")
                nc.vector.tensor_copy(out=c1[:], in_=ps1[:])
                c2 = wk.tile([P, P], f16, tag="c2")
                if has_b1:
                    nc.vector.tensor_add(out=c2[:], in0=ps2[:], in1=bres_t[:])
                else:
                    nc.scalar.activation(out=c2[:], in_=ps2[:], func=COPY)
                nc.sync.dma_start(out=xw1_own[t * P:(t + 1) * P, :], in_=c1[:])
                nc.sync.dma_start(out=xr03_dr[t * P:(t + 1) * P, :], in_=c2[:])
                if V_AGINLINE:
                    for j in range(NRANGE):
                        if t == qend[j]:
                            r0, r1 = QSTART[j] * P, (QSTART[j] + QB[j]) * P
                            nc.gpsimd.collective_compute(
                                "AllGather", mybir.AluOpType.bypass,
                                replica_groups=rg,
                                ins=[xw1_own[r0:r1, :]],
                                outs=[table1[RBASE[j]:RBASE[j] + RS[j], :]],
                            )
            if not V_AGINLINE:
                for j in range(NRANGE):
                    r0, r1 = QSTART[j] * P, (QSTART[j] + QB[j]) * P
                    nc.gpsimd.collective_compute(
                        "AllGather", mybir.AluOpType.bypass,
                        replica_groups=rg,
                        ins=[xw1_own[r0:r1, :]],
                        outs=[table1[RBASE[j]:RBASE[j] + RS[j], :]],
                    )

            # ---- aggregation pass: range-major gathers into zacc (fp16)
            nreg = nc.gpsimd.alloc_register("nidx")

            def agg_pass(table_d):
                for ci, (r, c0, ncall, groups) in enumerate(calls):
                    gb = gp.tile([P, MAXCH, D], f16, tag="g")
                    if V_REG:
                        nc.gpsimd.reg_load(nreg, ncnt_t[0:1, ci:ci + 1])
                    nc.gpsimd.dma_gather(
                        out_ap=gb[:, :ncall, :],
                        in_ap=table_d[RBASE[r]:RBASE[r] + RS[r], :],
                        idxs_ap=idx_t[:, 8 * c0:8 * (c0 + ncall)],
                        num_idxs=P * ncall,
                        num_idxs_reg=nreg if V_REG else P * ncall,
                        elem_size=D,
                        queue_num=qctr[0] % 4,
                    )
                    qctr[0] += 1
                    for (b, g0, kk) in groups:
                        sw = sp.tile([P, MAXCH, P], f16, tag="sw")
                        if V_BATCH:
                            io_b = iota_t[:].unsqueeze(1).broadcast_to([P, kk, P])
                            dst_b = dstl_t[:, g0:g0 + kk].unsqueeze(2).broadcast_to([P, kk, P])
                            nrm_b = norm_t[:, g0:g0 + kk].unsqueeze(2).broadcast_to([P, kk, P])
                            nc.vector.tensor_tensor(out=sw[:, :kk, :], in0=io_b, in1=dst_b, op=EQ)
                            nc.vector.tensor_tensor(out=sw[:, :kk, :], in0=sw[:, :kk, :], in1=nrm_b, op=MUL)
                        else:
                            for j in range(kk):
                                nc.vector.tensor_scalar(
                                    out=sw[:, j, :], in0=iota_t[:],
                                    scalar1=dstl_t[:, g0 + j:g0 + j + 1],
                                    scalar2=norm_t[:, g0 + j:g0 + j + 1],
                                    op0=EQ, op1=MUL,
                                )
                        zp = pz.tile([P, P], f32, space="PSUM", tag="z")
                        for j in range(kk):
                            nc.tensor.matmul(
                                zp[:], sw[:, j, :], gb[:, (g0 - c0) + j, :],
                                start=(j == 0), stop=(j == kk - 1),
                            )
                        zslice = zacc[:, b * P:(b + 1) * P]
                        if r == 0:
                            nc.vector.tensor_copy(out=zslice, in_=zp[:])
                        else:
                            nc.vector.tensor_tensor(out=zslice, in0=zslice, in1=zp[:], op=ADD)

            if V_STAGE == "A":
                for b in range(NBLK):
                    og = wk.tile([P, P], f32, tag="ge")
                    nc.vector.tensor_copy(out=og[:], in_=zacc[:, b * P:(b + 1) * P])
                    nc.sync.dma_start(out=out_d[b * P:(b + 1) * P, :], in_=og[:])
            if V_STAGE != "A":
                agg_pass(table1)

            if V_STAGE == "A1":
                for b in range(NBLK):
                    og = wk.tile([P, P], f32, tag="ge")
                    nc.vector.tensor_copy(out=og[:], in_=zacc[:, b * P:(b + 1) * P])
                    nc.sync.dma_start(out=out_d[b * P:(b + 1) * P, :], in_=og[:])

            # ---- layer-1 epilogue + fused phase C (xw2 = h @ W2) + AG2
            for b in (range(NBLK) if V_STAGE in ("A1E", "FULL") else []):
                ob = wk.tile([P, P], f16, tag="ob")
                nc.sync.dma_start(out=ob[:], in_=xw1_own[b * P:(b + 1) * P, :])
                e2 = wk.tile([P, P], f32, tag="e2")
                nc.vector.scalar_tensor_tensor(
                    out=e2[:], in0=ob[:], scalar=dinv2_t[:, b:b + 1],
                    in1=zacc[:, b * P:(b + 1) * P], op0=MUL, op1=ADD,
                )
                if has_b1:
                    nc.vector.tensor_add(out=e2[:], in0=e2[:], in1=bias1_t[:])
                ge = wk.tile([P, P], f32, tag="ge")
                nc.scalar.activation(out=ge[:], in_=e2[:], func=GELU)
                xr = wk.tile([P, P], f16, tag="xr")
                nc.sync.dma_start(out=xr[:], in_=xr03_dr[b * P:(b + 1) * P, :])
                hb = wk.tile([P, P], f32, tag="hb")
                nc.vector.tensor_tensor(out=hb[:], in0=ge[:], in1=xr[:], op=ADD)
                pt = pa.tile([P, P], f32, space="PSUM", tag="ps2")
                nc.tensor.transpose(out=pt[:], in_=hb[:], identity=id_t[:])
                hc = wk.tile([P, P], f16, tag="hc")
                nc.vector.tensor_copy(out=hc[:], in_=pt[:])
                psC = pa.tile([P, P], f32, space="PSUM", tag="ps1")
                nc.tensor.matmul(psC[:], hc[:], w2_t[:], start=True, stop=True)
                cC = wk.tile([P, P], f16, tag="cC")
                nc.scalar.activation(out=cC[:], in_=psC[:], func=COPY)
                nc.sync.dma_start(out=xw2_own[b * P:(b + 1) * P, :], in_=cC[:])
                if V_STAGE == "A1E":
                    nc.sync.dma_start(out=out_d[b * P:(b + 1) * P, :], in_=ge[:])
                if V_AGINLINE:
                    for j in range(NRANGE):
                        if b == qend[j]:
                            r0, r1 = QSTART[j] * P, (QSTART[j] + QB[j]) * P
                            nc.gpsimd.collective_compute(
                                "AllGather", mybir.AluOpType.bypass,
                                replica_groups=rg,
                                ins=[xw2_own[r0:r1, :]],
                                outs=[table2[RBASE[j]:RBASE[j] + RS[j], :]],
                            )
            if not V_AGINLINE:
                for j in range(NRANGE):
                    r0, r1 = QSTART[j] * P, (QSTART[j] + QB[j]) * P
                    nc.gpsimd.collective_compute(
                        "AllGather", mybir.AluOpType.bypass,
                        replica_groups=rg,
                        ins=[xw2_own[r0:r1, :]],
                        outs=[table2[RBASE[j]:RBASE[j] + RS[j], :]],
                    )

            if V_STAGE == "FULL":
                agg_pass(table2)

            # ---- layer-2 epilogue
            for b in (range(NBLK) if V_STAGE == "FULL" else []):
                ob = wk.tile([P, P], f16, tag="ob")
                nc.sync.dma_start(out=ob[:], in_=xw2_own[b * P:(b + 1) * P, :])
                e2 = wk.tile([P, P], f32, tag="e2")
                nc.vector.scalar_tensor_tensor(
                    out=e2[:], in0=ob[:], scalar=dinv2_t[:, b:b + 1],
                    in1=zacc[:, b * P:(b + 1) * P], op0=MUL, op1=ADD,
                )
                if has_b2:
                    nc.vector.tensor_add(out=e2[:], in0=e2[:], in1=bias2_t[:])
                og = wk.tile([P, P], f32, tag="ge")
                nc.scalar.activation(out=og[:], in_=e2[:], func=GELU)
                nc.sync.dma_start(out=out_d[b * P:(b + 1) * P, :], in_=og[:])

    nc.compile()
    return nc


def _get_compiled(C, calls, has_b1, has_b2):
    key = (C, calls, has_b1, has_b2, V_AGINLINE, V_REG, V_BATCH, V_STAGE)
    if key not in _CACHE:
        _CACHE[key] = _build(C, calls, has_b1, has_b2)
    return _CACHE[key]


def kernel(x, edge_index, B, N, causal_edge_index, edge_weight,
           causal_edge_weight, W1, b1, W2, b2, Wres, bres):
    assert int(B) * int(N) == NCORES * NPC
    from concourse.bass_utils import run_bass_kernel_spmd

    C, calls, has_b1, has_b2, in_maps = _preprocess(
        x, edge_index, edge_weight, W1, b1, W2, b2, Wres, bres)
    nc = _get_compiled(C, calls, has_b1, has_b2)
    res = run_bass_kernel_spmd(nc, in_maps, list(range(NCORES)))
    out = np.concatenate(
        [res.results[c]["out"][:NPC] for c in range(NCORES)], axis=0)
    return out.astype(np.float32)


# exposed for test.py so it can reuse preprocessing + run with tracing
def _run_traced(x, edge_index, edge_weight, W1, b1, W2, b2, Wres, bres,
                **trace_kwargs):
    from concourse.bass_utils import run_bass_kernel_spmd
    C, calls, has_b1, has_b2, in_maps = _preprocess(
        x, edge_index, edge_weight, W1, b1, W2, b2, Wres, bres)
    nc = _get_compiled(C, calls, has_b1, has_b2)
    res = run_bass_kernel_spmd(nc, in_maps, list(range(NCORES)),
                               **trace_kwargs)
    out = np.concatenate(
        [res.results[c]["out"][:NPC] for c in range(NCORES)], axis=0)
    return out.astype(np.float32), res


# revision 14
# speedup vs baseline: 2.3321x; 1.0355x over previous
"""Two-layer GCN (PyG GCNConv x2 + gelu + scaled residual) on 8 trn2 NeuronCores.

Strategy (per the sharding hint):
  - Nodes partitioned contiguously across the 8 cores (12500 each); edges
    assigned to the core owning their destination node.
  - 128x128 weights replicated; per-layer node-feature tables (xw = x @ W)
    are computed shard-wise and AllGathered (in 4 pipelined quarter
    collectives) so each core can gather the rows of its edges' source
    nodes ("halo exchange").
  - Aggregation: per dst-block (128 nodes), gather y[src] rows in fp16 via
    SWDGE dma_gather (4 queues round-robin = 4 Q7 core pairs), build a
    weighted one-hot selection S_w[k, j] = norm[k] * (dst_local[k] == j)
    with batched broadcast-AP tensor_tensor ops, and accumulate
    z += S_w.T @ G on the tensor engine (fp16 in, fp32 PSUM accumulate).
  - Range-major processing with an SBUF fp16 accumulator so each table
    quarter is consumed as soon as its collective lands.
  - Degree/normalization and edge->slot layout are host-side preprocessing
    of the static graph structure. Trailing padding slots use idx=-1 which
    the gather ucode truncates (no descriptor cost).

Math:
  gcn(x, W, b) = dinv * (segsum_dst(w_e * y[src]) + y[i]) + b,
      where y = (x @ W) * dinv, dinv = rsqrt(deg + 1)
  equivalently with host-computed norm_e = dinv[src]*w*dinv[dst]:
      agg[i] = segsum_dst(norm_e * xw[src]) + dinv2[i]*xw[i] + b
  h   = gelu(agg1) + (x @ (0.3*Wres) + 0.3*bres)
  out = gelu(agg2(h))
"""

import os

import numpy as np

V_AGINLINE = os.environ.get("V_AGINLINE", "1") == "1"   # quarter-AGs inline vs at phase end
V_REG = os.environ.get("V_REG", "1") == "1"             # per-core num_idxs_reg + trailing -1
V_BATCH = os.environ.get("V_BATCH", "1") == "1"         # batched broadcast-AP S_w builds
V_STAGE = os.environ.get("V_STAGE", "FULL")             # A | A1 | A1E | FULL

P = 128
D = 128
NCORES = 8
NPC = 12500          # nodes per core
NBLK = 98            # 128-node blocks per core (98*128 = 12544)
NPCP = NBLK * P      # padded nodes per core
NRANGE = 4
QB = [25, 25, 24, 24]            # blocks per quarter (sum = 98)
QSTART = [0, 25, 50, 74]         # first block of each quarter
QROWS = [q * P for q in QB]      # rows per quarter per core
RS = [NCORES * qr for qr in QROWS]   # table rows per range
RBASE = [0, RS[0], RS[0] + RS[1], RS[0] + RS[1] + RS[2]]
TR = sum(RS)         # table rows (100352)
MAXCH = 8            # max chunks per dma_gather call (1024 idxs; ring cap)

_CACHE = {}


def _preprocess(x, edge_index, edge_weight, W1, b1, W2, b2, Wres, bres):
    BN = NCORES * NPC
    src = np.asarray(edge_index[0], dtype=np.int64)
    dst = np.asarray(edge_index[1], dtype=np.int64)
    w = np.asarray(edge_weight, dtype=np.float64)

    deg = np.bincount(dst, weights=w, minlength=BN) + 1.0
    dinv = 1.0 / np.sqrt(deg)
    norm_e = (dinv[src] * w * dinv[dst]).astype(np.float32)
    dinv2 = (dinv * dinv).astype(np.float32)

    # --- source node -> (range, ridx) via quarter-major table layout
    s_core = src // NPC
    s_loc = src - s_core * NPC
    s_blk = s_loc // P
    qof = np.zeros(NBLK, np.int64)
    for q in range(NRANGE):
        qof[QSTART[q]:QSTART[q] + QB[q]] = q
    rng = qof[s_blk]
    qstart_rows = np.array([QSTART[q] * P for q in range(NRANGE)])
    qrows = np.array(QROWS)
    ridx = (s_core * qrows[rng] + (s_loc - qstart_rows[rng])).astype(np.int16)

    # --- destination -> (core, block, dst-local)
    core = dst // NPC
    loc = dst - core * NPC
    blk = loc // P
    dl = (loc % P).astype(np.float32)

    # --- group edges by (core, rng, blk); stable order within groups
    order = np.lexsort((blk, rng, core))
    core_s, blk_s, rng_s = core[order], blk[order], rng[order]
    gid = (core_s * NRANGE + rng_s) * NBLK + blk_s
    ngroups = NCORES * NRANGE * NBLK
    cnt = np.bincount(gid, minlength=ngroups)
    start = np.concatenate([[0], np.cumsum(cnt)[:-1]])
    q = np.arange(len(gid)) - start[gid]

    # static chunk structure: K[r, b] = max over cores
    cnt3 = cnt.reshape(NCORES, NRANGE, NBLK)
    K = np.maximum(np.ceil(cnt3 / P).astype(np.int64).max(axis=0), 1)  # [NRANGE, NBLK]

    # chunk column bases, range-major then block
    cbase = np.zeros((NRANGE, NBLK), np.int64)
    run = 0
    for r in range(NRANGE):
        for b in range(NBLK):
            cbase[r, b] = run
            run += K[r, b]
    C = int(run)

    # gather call packing: per range, greedy consecutive groups, <= MAXCH chunks
    # call = (r, c0, ncall, groups=[(b, gcol0, kk), ...])
    calls = []
    for r in range(NRANGE):
        b = 0
        while b < NBLK:
            c0 = int(cbase[r, b])
            ncall = 0
            groups = []
            while b < NBLK and ncall + int(K[r, b]) <= MAXCH:
                groups.append((b, int(cbase[r, b]), int(K[r, b])))
                ncall += int(K[r, b])
                b += 1
            calls.append((r, c0, ncall, groups))

    # per-edge slot: chunk col cc, partition pp
    cc = cbase[rng_s, blk_s] + q // P
    pp = q % P
    flat = cc * P + pp  # flat slot id in [0, C*P)

    slot_idx = np.zeros((NCORES, C * P), np.int16)
    slot_dstl = np.zeros((NCORES, C * P), np.float32)
    slot_norm = np.zeros((NCORES, C * P), np.float32)
    slot_idx[core_s, flat] = ridx[order]
    slot_dstl[core_s, flat] = dl[order]
    slot_norm[core_s, flat] = norm_e[order]

    # Per-(core, call) runtime gather count: whole trailing padding chunks are
    # marked -1 (the gather ucode truncates trailing negatives) and the count
    # register is rounded to the same 128 boundary so the decode-side ring
    # reservation matches the ucode's descriptor count exactly. Partial-chunk
    # padding keeps idx 0 (dummy descriptors cost the same either way).
    real = np.zeros((NCORES, C * P), bool)
    real[core_s, flat] = True
    ncalls_pad = -(-len(calls) // 4) * 4
    nidx_call = np.zeros((NCORES, ncalls_pad), np.int32)
    for ci, (_r, c0, ncall, _g) in enumerate(calls):
        s0, s1 = c0 * P, (c0 + ncall) * P
        seg = real[:, s0:s1]
        rev = seg[:, ::-1]
        has = rev.any(axis=1)
        last = np.where(has, (s1 - s0 - 1) - rev.argmax(axis=1), -1)
        kept = (last // P + 1) * P          # 128-rounded per-core count
        nidx_call[:, ci] = kept
        if V_REG:
            mask = np.arange(s1 - s0)[None, :] >= kept[:, None]
            slot_idx[:, s0:s1][mask] = -1

    # idx16 layout [16, 8*C] wrapped, replicated to 128 partitions
    arr = slot_idx.reshape(NCORES, C, 8, 16)            # [c, cc, pp//16, pp%16]
    idx16s = arr.transpose(0, 3, 1, 2).reshape(NCORES, 16, 8 * C)
    idx16 = np.tile(idx16s, (1, 8, 1))                  # [c, 128, 8C]

    dstl = slot_dstl.reshape(NCORES, C, P).transpose(0, 2, 1).astype(np.float16)
    normv = slot_norm.reshape(NCORES, C, P).transpose(0, 2, 1).astype(np.float16)
    normf = slot_norm.reshape(NCORES, C, P).transpose(0, 2, 1).astype(np.float32)

    d2 = np.zeros((NCORES, NPCP), np.float32)
    for c in range(NCORES):
        d2[c, :NPC] = dinv2[c * NPC:(c + 1) * NPC]
    dinv2o = d2.reshape(NCORES, NBLK, P).transpose(0, 2, 1).copy()

    xT = np.zeros((NCORES, P, NPCP), np.float16)
    xf = np.asarray(x, dtype=np.float32)
    for c in range(NCORES):
        xT[c, :, :NPC] = xf[c * NPC:(c + 1) * NPC].T.astype(np.float16)

    iota = np.broadcast_to(np.arange(P, dtype=np.float16), (P, P)).copy()
    ident = np.eye(P, dtype=np.float32)

    consts = dict(
        W1=np.asarray(W1, np.float32).astype(np.float16),
        W2=np.asarray(W2, np.float32).astype(np.float16),
        Wres03=(0.3 * np.asarray(Wres, np.float32)).astype(np.float16),
        iota=iota, ident=ident,
    )
    b1 = np.asarray(b1, np.float32)
    b2 = np.asarray(b2, np.float32)
    bres03 = 0.3 * np.asarray(bres, np.float32)
    has_b1 = bool(np.any(b1)) or bool(np.any(bres03))
    has_b2 = bool(np.any(b2))
    if has_b1:
        consts["bias1"] = np.broadcast_to(b1, (P, P)).copy()
        consts["bres03"] = np.broadcast_to(bres03, (P, P)).copy()
    if has_b2:
        consts["bias2"] = np.broadcast_to(b2, (P, P)).copy()

    in_maps = []
    for c in range(NCORES):
        m = dict(consts)
        m.update(xT=xT[c], idx16=idx16[c], dstl=dstl[c], normv=normv[c],
                 dinv2o=dinv2o[c], ncnt=nidx_call[c:c + 1], normf=normf[c])
        in_maps.append(m)
    call_sig = tuple((r, c0, ncall, tuple(g)) for r, c0, ncall, g in calls)
    return C, call_sig, has_b1, has_b2, in_maps


def _build(C, calls, has_b1, has_b2):
    import concourse.bacc as bacc
    import concourse.bass as bass
    import concourse.mybir as mybir
    import concourse.tile as tile

    f32 = mybir.dt.float32
    f16 = mybir.dt.float16
    i16 = mybir.dt.int16
    EQ = mybir.AluOpType.is_equal
    MUL = mybir.AluOpType.mult
    ADD = mybir.AluOpType.add
    GELU = mybir.ActivationFunctionType.Gelu
    COPY = mybir.ActivationFunctionType.Copy

    nc = bacc.Bacc(num_swdge_queues=4)

    xT_d = nc.dram_tensor("xT", [P, NPCP], f16, kind="ExternalInput")
    W1_d = nc.dram_tensor("W1", [P, P], f16, kind="ExternalInput")
    W2_d = nc.dram_tensor("W2", [P, P], f16, kind="ExternalInput")
    Wres_d = nc.dram_tensor("Wres03", [P, P], f16, kind="ExternalInput")
    iota_d = nc.dram_tensor("iota", [P, P], f16, kind="ExternalInput")
    ident_d = nc.dram_tensor("ident", [P, P], f32, kind="ExternalInput")
    idx_d = nc.dram_tensor("idx16", [P, 8 * C], i16, kind="ExternalInput")
    dstl_d = nc.dram_tensor("dstl", [P, C], f16, kind="ExternalInput")
    norm_d = nc.dram_tensor("normv", [P, C], f16, kind="ExternalInput")
    dinv2_d = nc.dram_tensor("dinv2o", [P, NBLK], f32, kind="ExternalInput")
    ncalls_pad = -(-len(calls) // 4) * 4
    ncnt_d = nc.dram_tensor("ncnt", [1, ncalls_pad], mybir.dt.int32,
                            kind="ExternalInput")
    normf_d = nc.dram_tensor("normf", [P, C], f32, kind="ExternalInput")
    bias1_d = nc.dram_tensor("bias1", [P, P], f32, kind="ExternalInput") if has_b1 else None
    bres_d = nc.dram_tensor("bres03", [P, P], f32, kind="ExternalInput") if has_b1 else None
    bias2_d = nc.dram_tensor("bias2", [P, P], f32, kind="ExternalInput") if has_b2 else None

    out_d = nc.dram_tensor("out", [NPCP, D], f32, kind="ExternalOutput")

    xw1_own = nc.dram_tensor("xw1_own", [NPCP, D], f16)
    xr03_dr = nc.dram_tensor("xr03", [NPCP, D], f16)
    xw2_own = nc.dram_tensor("xw2_own", [NPCP, D], f16)
    table1 = nc.dram_tensor("table1", [TR, D], f16, addr_space="Shared")
    table2 = nc.dram_tensor("table2", [TR, D], f16, addr_space="Shared")

    rg = [list(range(NCORES))]
    GBUFS = 12
    qctr = [0]

    with tile.TileContext(nc) as tc:
        with (
            tc.tile_pool(name="meta", bufs=1) as mp,
            tc.tile_pool(name="gp", bufs=GBUFS) as gp,
            tc.tile_pool(name="sp", bufs=6) as sp,
            tc.tile_pool(name="wk", bufs=3) as wk,
            tc.tile_pool(name="pz", bufs=3, space="PSUM") as pz,
            tc.tile_pool(name="pa", bufs=2, space="PSUM") as pa,
        ):
            # ---- resident tiles
            w1_t = mp.tile([P, P], f16)
            w2_t = mp.tile([P, P], f16)
            wr_t = mp.tile([P, P], f16)
            iota_t = mp.tile([P, P], f16)
            id_t = mp.tile([P, P], f32)
            idx_t = mp.tile([P, 8 * C], i16)
            dstl_t = mp.tile([P, C], f16)
            norm_t = mp.tile([P, C], f16)
            dinv2_t = mp.tile([P, NBLK], f32)
            zacc = mp.tile([P, NBLK * P], f16)
            ncnt_t = mp.tile([1, ncalls_pad], mybir.dt.int32)
            nc.sync.dma_start(out=ncnt_t[:], in_=ncnt_d[:])
            normf_t = mp.tile([P, C], f32)
            nc.sync.dma_start(out=normf_t[:], in_=normf_d[:])
            nc.sync.dma_start(out=w1_t[:], in_=W1_d[:])
            nc.sync.dma_start(out=w2_t[:], in_=W2_d[:])
            nc.sync.dma_start(out=wr_t[:], in_=Wres_d[:])
            nc.sync.dma_start(out=iota_t[:], in_=iota_d[:])
            nc.sync.dma_start(out=id_t[:], in_=ident_d[:])
            nc.sync.dma_start(out=idx_t[:], in_=idx_d[:])
            nc.sync.dma_start(out=dstl_t[:], in_=dstl_d[:])
            nc.sync.dma_start(out=norm_t[:], in_=norm_d[:])
            nc.sync.dma_start(out=dinv2_t[:], in_=dinv2_d[:])
            if has_b1:
                bias1_t = mp.tile([P, P], f32)
                bres_t = mp.tile([P, P], f32)
                nc.sync.dma_start(out=bias1_t[:], in_=bias1_d[:])
                nc.sync.dma_start(out=bres_t[:], in_=bres_d[:])
            if has_b2:
                bias2_t = mp.tile([P, P], f32)
                nc.sync.dma_start(out=bias2_t[:], in_=bias2_d[:])

            # zero the gather buffers once: truncated gather slots leave SBUF
            # untouched, and the matmul multiplies them by 0 — stale NaNs from
            # uninitialized SBUF would poison 0*NaN. After this, reuses only
            # ever contain previous gathered table rows (finite).
            for _ in range(GBUFS):
                g0 = gp.tile([P, MAXCH, D], f16, tag="g")
                nc.vector.memset(g0[:], 0)

            # ---- phase A: xw1 = x@W1, xr03 = x@(0.3*Wres), shard-local;
            # fire quarter-AllGathers as their blocks complete.
            qend = [QSTART[j] + QB[j] - 1 for j in range(NRANGE)]
            for t in range(NBLK):
                xt = wk.tile([P, P], f16, tag="xt")
                nc.sync.dma_start(out=xt[:], in_=xT_d[:, t * P:(t + 1) * P])
                ps1 = pa.tile([P, P], f32, space="PSUM", tag="ps1")
                ps2 = pa.tile([P, P], f32, space="PSUM", tag="ps2")
                nc.tensor.matmul(ps1[:], xt[:], w1_t[:], start=True, stop=True)
                nc.tensor.matmul(ps2[:], xt[:], wr_t[:], start=True, stop=True)
                c1 = wk.tile([P, P], f16, tag="c1")
                nc.vector.tensor_copy(out=c1[:], in_=ps1[:])
                c2 = wk.tile([P, P], f16, tag="c2")
                if has_b1:
                    nc.vector.tensor_add(out=c2[:], in0=ps2[:], in1=bres_t[:])
                else:
                    nc.scalar.activation(out=c2[:], in_=ps2[:], func=COPY)
                nc.sync.dma_start(out=xw1_own[t * P:(t + 1) * P, :], in_=c1[:])
                nc.sync.dma_start(out=xr03_dr[t * P:(t + 1) * P, :], in_=c2[:])
                if V_AGINLINE:
                    for j in range(NRANGE):
                        if t == qend[j]:
                            r0, r1 = QSTART[j] * P, (QSTART[j] + QB[j]) * P
                            nc.gpsimd.collective_compute(
                                "AllGather", mybir.AluOpType.bypass,
                                replica_groups=rg,
                                ins=[xw1_own[r0:r1, :]],
                                outs=[table1[RBASE[j]:RBASE[j] + RS[j], :]],
                            )
            if not V_AGINLINE:
                for j in range(NRANGE):
                    r0, r1 = QSTART[j] * P, (QSTART[j] + QB[j]) * P
                    nc.gpsimd.collective_compute(
                        "AllGather", mybir.AluOpType.bypass,
                        replica_groups=rg,
                        ins=[xw1_own[r0:r1, :]],
                        outs=[table1[RBASE[j]:RBASE[j] + RS[j], :]],
                    )

            # ---- aggregation pass: range-major gathers into zacc (fp16)
            nregs = [nc.gpsimd.alloc_register(f"nidx{i}") for i in range(4)]

            def agg_pass(table_d):
                for ci, (r, c0, ncall, groups) in enumerate(calls):
                    gb = gp.tile([P, MAXCH, D], f16, tag="g")
                    if V_REG and ci % 4 == 0:
                        nc.gpsimd.reg_load(nregs, ncnt_t[0:1, ci:ci + 4])
                    nc.gpsimd.dma_gather(
                        out_ap=gb[:, :ncall, :],
                        in_ap=table_d[RBASE[r]:RBASE[r] + RS[r], :],
                        idxs_ap=idx_t[:, 8 * c0:8 * (c0 + ncall)],
                        num_idxs=P * ncall,
                        num_idxs_reg=nregs[ci % 4] if V_REG else P * ncall,
                        elem_size=D,
                        queue_num=qctr[0] % 4,
                    )
                    qctr[0] += 1
                    for (b, g0, kk) in groups:
                        sw = sp.tile([P, MAXCH, P], f16, tag="sw")
                        if V_BATCH:
                            io_b = iota_t[:].unsqueeze(1).broadcast_to([P, kk, P])
                            dst_b = dstl_t[:, g0:g0 + kk].unsqueeze(2).broadcast_to([P, kk, P])
                            nc.vector.tensor_tensor(out=sw[:, :kk, :], in0=io_b, in1=dst_b, op=EQ)
                            if qctr[0] % 2 == 0:
                                nrm_b = norm_t[:, g0:g0 + kk].unsqueeze(2).broadcast_to([P, kk, P])
                                nc.vector.tensor_tensor(out=sw[:, :kk, :], in0=sw[:, :kk, :], in1=nrm_b, op=MUL)
                            else:
                                for j in range(kk):
                                    nc.scalar.activation(
                                        out=sw[:, j, :], in_=sw[:, j, :], func=COPY,
                                        scale=normf_t[:, g0 + j:g0 + j + 1])
                        else:
                            for j in range(kk):
                                nc.vector.tensor_scalar(
                                    out=sw[:, j, :], in0=iota_t[:],
                                    scalar1=dstl_t[:, g0 + j:g0 + j + 1],
                                    scalar2=norm_t[:, g0 + j:g0 + j + 1],
                                    op0=EQ, op1=MUL,
                                )
                        zp = pz.tile([P, P], f32, space="PSUM", tag="z")
                        for j in range(kk):
                            nc.tensor.matmul(
                                zp[:], sw[:, j, :], gb[:, (g0 - c0) + j, :],
                                start=(j == 0), stop=(j == kk - 1),
                            )
                        zslice = zacc[:, b * P:(b + 1) * P]
                        if r == 0:
                            nc.scalar.activation(out=zslice, in_=zp[:], func=COPY)
                        else:
                            nc.vector.tensor_tensor(out=zslice, in0=zslice, in1=zp[:], op=ADD)

            if V_STAGE == "A":
                for b in range(NBLK):
                    og = wk.tile([P, P], f32, tag="ge")
                    nc.vector.tensor_copy(out=og[:], in_=zacc[:, b * P:(b + 1) * P])
                    nc.sync.dma_start(out=out_d[b * P:(b + 1) * P, :], in_=og[:])
            if V_STAGE != "A":
                agg_pass(table1)

            if V_STAGE == "A1":
                for b in range(NBLK):
                    og = wk.tile([P, P], f32, tag="ge")
                    nc.vector.tensor_copy(out=og[:], in_=zacc[:, b * P:(b + 1) * P])
                    nc.sync.dma_start(out=out_d[b * P:(b + 1) * P, :], in_=og[:])

            # ---- layer-1 epilogue + fused phase C (xw2 = h @ W2) + AG2
            for b in (range(NBLK) if V_STAGE in ("A1E", "FULL") else []):
                ob = wk.tile([P, P], f16, tag="ob")
                nc.sync.dma_start(out=ob[:], in_=xw1_own[b * P:(b + 1) * P, :])
                e2 = wk.tile([P, P], f32, tag="e2")
                nc.vector.scalar_tensor_tensor(
                    out=e2[:], in0=ob[:], scalar=dinv2_t[:, b:b + 1],
                    in1=zacc[:, b * P:(b + 1) * P], op0=MUL, op1=ADD,
                )
                if has_b1:
                    nc.vector.tensor_add(out=e2[:], in0=e2[:], in1=bias1_t[:])
                ge = wk.tile([P, P], f32, tag="ge")
                nc.scalar.activation(out=ge[:], in_=e2[:], func=GELU)
                xr = wk.tile([P, P], f16, tag="xr")
                nc.sync.dma_start(out=xr[:], in_=xr03_dr[b * P:(b + 1) * P, :])
                hb = wk.tile([P, P], f32, tag="hb")
                nc.vector.tensor_tensor(out=hb[:], in0=ge[:], in1=xr[:], op=ADD)
                pt = pa.tile([P, P], f32, space="PSUM", tag="ps2")
                nc.tensor.transpose(out=pt[:], in_=hb[:], identity=id_t[:])
                hc = wk.tile([P, P], f16, tag="hc")
                nc.scalar.activation(out=hc[:], in_=pt[:], func=COPY)
                psC = pa.tile([P, P], f32, space="PSUM", tag="ps1")
                nc.tensor.matmul(psC[:], hc[:], w2_t[:], start=True, stop=True)
                cC = wk.tile([P, P], f16, tag="cC")
                nc.scalar.activation(out=cC[:], in_=psC[:], func=COPY)
                nc.sync.dma_start(out=xw2_own[b * P:(b + 1) * P, :], in_=cC[:])
                if V_STAGE == "A1E":
                    nc.sync.dma_start(out=out_d[b * P:(b + 1) * P, :], in_=ge[:])
                if V_AGINLINE:
                    for j in range(NRANGE):
                        if b == qend[j]:
                            r0, r1 = QSTART[j] * P, (QSTART[j] + QB[j]) * P
                            nc.gpsimd.collective_compute(
                                "AllGather", mybir.AluOpType.bypass,
                                replica_groups=rg,
                                ins=[xw2_own[r0:r1, :]],
                                outs=[table2[RBASE[j]:RBASE[j] + RS[j], :]],
                            )
            if not V_AGINLINE:
                for j in range(NRANGE):
                    r0, r1 = QSTART[j] * P, (QSTART[j] + QB[j]) * P
                    nc.gpsimd.collective_compute(
                        "AllGather", mybir.AluOpType.bypass,
                        replica_groups=rg,
                        ins=[xw2_own[r0:r1, :]],
                        outs=[table2[RBASE[j]:RBASE[j] + RS[j], :]],
                    )

            if V_STAGE == "FULL":
                agg_pass(table2)

            # ---- layer-2 epilogue
            for b in (range(NBLK) if V_STAGE == "FULL" else []):
                ob = wk.tile([P, P], f16, tag="ob")
                nc.sync.dma_start(out=ob[:], in_=xw2_own[b * P:(b + 1) * P, :])
                e2 = wk.tile([P, P], f32, tag="e2")
                nc.vector.scalar_tensor_tensor(
                    out=e2[:], in0=ob[:], scalar=dinv2_t[:, b:b + 1],
                    in1=zacc[:, b * P:(b + 1) * P], op0=MUL, op1=ADD,
                )
                if has_b2:
                    nc.vector.tensor_add(out=e2[:], in0=e2[:], in1=bias2_t[:])
                og = wk.tile([P, P], f32, tag="ge")
                nc.scalar.activation(out=og[:], in_=e2[:], func=GELU)
                nc.sync.dma_start(out=out_d[b * P:(b + 1) * P, :], in_=og[:])

    nc.compile()
    return nc


def _get_compiled(C, calls, has_b1, has_b2):
    key = (C, calls, has_b1, has_b2, V_AGINLINE, V_REG, V_BATCH, V_STAGE)
    if key not in _CACHE:
        _CACHE[key] = _build(C, calls, has_b1, has_b2)
    return _CACHE[key]


def kernel(x, edge_index, B, N, causal_edge_index, edge_weight,
           causal_edge_weight, W1, b1, W2, b2, Wres, bres):
    assert int(B) * int(N) == NCORES * NPC
    from concourse.bass_utils import run_bass_kernel_spmd

    C, calls, has_b1, has_b2, in_maps = _preprocess(
        x, edge_index, edge_weight, W1, b1, W2, b2, Wres, bres)
    nc = _get_compiled(C, calls, has_b1, has_b2)
    res = run_bass_kernel_spmd(nc, in_maps, list(range(NCORES)))
    out = np.concatenate(
        [res.results[c]["out"][:NPC] for c in range(NCORES)], axis=0)
    return out.astype(np.float32)


# exposed for test.py so it can reuse preprocessing + run with tracing
def _run_traced(x, edge_index, edge_weight, W1, b1, W2, b2, Wres, bres,
                **trace_kwargs):
    from concourse.bass_utils import run_bass_kernel_spmd
    C, calls, has_b1, has_b2, in_maps = _preprocess(
        x, edge_index, edge_weight, W1, b1, W2, b2, Wres, bres)
    nc = _get_compiled(C, calls, has_b1, has_b2)
    res = run_bass_kernel_spmd(nc, in_maps, list(range(NCORES)),
                               **trace_kwargs)
    out = np.concatenate(
        [res.results[c]["out"][:NPC] for c in range(NCORES)], axis=0)
    return out.astype(np.float32), res


# revision 15
# speedup vs baseline: 2.5078x; 1.0753x over previous
"""Two-layer GCN (PyG GCNConv x2 + gelu + scaled residual) on 8 trn2 NeuronCores.

Strategy (per the sharding hint):
  - Nodes partitioned contiguously across the 8 cores (12500 each); edges
    assigned to the core owning their destination node.
  - 128x128 weights replicated; per-layer node-feature tables (xw = x @ W)
    are computed shard-wise and AllGathered (in 4 pipelined quarter
    collectives) so each core can gather the rows of its edges' source
    nodes ("halo exchange").
  - Aggregation: per dst-block (128 nodes), gather y[src] rows in fp16 via
    SWDGE dma_gather (4 queues round-robin = 4 Q7 core pairs), build a
    weighted one-hot selection S_w[k, j] = norm[k] * (dst_local[k] == j)
    with batched broadcast-AP tensor_tensor ops, and accumulate
    z += S_w.T @ G on the tensor engine (fp16 in, fp32 PSUM accumulate).
  - Range-major processing with an SBUF fp16 accumulator so each table
    quarter is consumed as soon as its collective lands.
  - Degree/normalization and edge->slot layout are host-side preprocessing
    of the static graph structure. Trailing padding slots use idx=-1 which
    the gather ucode truncates (no descriptor cost).

Math:
  gcn(x, W, b) = dinv * (segsum_dst(w_e * y[src]) + y[i]) + b,
      where y = (x @ W) * dinv, dinv = rsqrt(deg + 1)
  equivalently with host-computed norm_e = dinv[src]*w*dinv[dst]:
      agg[i] = segsum_dst(norm_e * xw[src]) + dinv2[i]*xw[i] + b
  h   = gelu(agg1) + (x @ (0.3*Wres) + 0.3*bres)
  out = gelu(agg2(h))
"""

import os

import numpy as np

V_AGINLINE = os.environ.get("V_AGINLINE", "1") == "1"   # quarter-AGs inline vs at phase end
V_REG = os.environ.get("V_REG", "1") == "1"             # per-core num_idxs_reg + trailing -1
V_BATCH = os.environ.get("V_BATCH", "1") == "1"         # batched broadcast-AP S_w builds
V_STAGE = os.environ.get("V_STAGE", "FULL")             # A | A1 | A1E | FULL

P = 128
D = 128
NCORES = 8
NPC = 12500          # nodes per core
NBLK = 98            # 128-node blocks per core (98*128 = 12544)
NPCP = NBLK * P      # padded nodes per core
NRANGE = 4
QB = [13, 29, 28, 28]            # blocks per quarter (sum = 98; small q0 so AG_0 fires early)
QSTART = [0, 13, 42, 70]         # first block of each quarter
QROWS = [q * P for q in QB]      # rows per quarter per core
RS = [NCORES * qr for qr in QROWS]   # table rows per range
RBASE = [0, RS[0], RS[0] + RS[1], RS[0] + RS[1] + RS[2]]
TR = sum(RS)         # table rows (100352)
MAXCH = 8            # max chunks per dma_gather call (1024 idxs; ring cap)

_CACHE = {}


def _preprocess(x, edge_index, edge_weight, W1, b1, W2, b2, Wres, bres):
    BN = NCORES * NPC
    src = np.asarray(edge_index[0], dtype=np.int64)
    dst = np.asarray(edge_index[1], dtype=np.int64)
    w = np.asarray(edge_weight, dtype=np.float64)

    deg = np.bincount(dst, weights=w, minlength=BN) + 1.0
    dinv = 1.0 / np.sqrt(deg)
    norm_e = (dinv[src] * w * dinv[dst]).astype(np.float32)
    dinv2 = (dinv * dinv).astype(np.float32)

    # --- source node -> (range, ridx) via quarter-major table layout
    s_core = src // NPC
    s_loc = src - s_core * NPC
    s_blk = s_loc // P
    qof = np.zeros(NBLK, np.int64)
    for q in range(NRANGE):
        qof[QSTART[q]:QSTART[q] + QB[q]] = q
    rng = qof[s_blk]
    qstart_rows = np.array([QSTART[q] * P for q in range(NRANGE)])
    qrows = np.array(QROWS)
    ridx = (s_core * qrows[rng] + (s_loc - qstart_rows[rng])).astype(np.int16)

    # --- destination -> (core, block, dst-local)
    core = dst // NPC
    loc = dst - core * NPC
    blk = loc // P
    dl = (loc % P).astype(np.float32)

    # --- group edges by (core, rng, blk); stable order within groups
    order = np.lexsort((blk, rng, core))
    core_s, blk_s, rng_s = core[order], blk[order], rng[order]
    gid = (core_s * NRANGE + rng_s) * NBLK + blk_s
    ngroups = NCORES * NRANGE * NBLK
    cnt = np.bincount(gid, minlength=ngroups)
    start = np.concatenate([[0], np.cumsum(cnt)[:-1]])
    q = np.arange(len(gid)) - start[gid]

    # static chunk structure: K[r, b] = max over cores
    cnt3 = cnt.reshape(NCORES, NRANGE, NBLK)
    K = np.maximum(np.ceil(cnt3 / P).astype(np.int64).max(axis=0), 1)  # [NRANGE, NBLK]

    # chunk column bases, range-major then block
    cbase = np.zeros((NRANGE, NBLK), np.int64)
    run = 0
    for r in range(NRANGE):
        for b in range(NBLK):
            cbase[r, b] = run
            run += K[r, b]
    C = int(run)

    # gather call packing: per range, greedy consecutive groups, <= MAXCH chunks
    # call = (r, c0, ncall, groups=[(b, gcol0, kk), ...])
    calls = []
    for r in range(NRANGE):
        b = 0
        while b < NBLK:
            c0 = int(cbase[r, b])
            ncall = 0
            groups = []
            while b < NBLK and ncall + int(K[r, b]) <= MAXCH:
                groups.append((b, int(cbase[r, b]), int(K[r, b])))
                ncall += int(K[r, b])
                b += 1
            calls.append((r, c0, ncall, groups))

    # per-edge slot: chunk col cc, partition pp
    cc = cbase[rng_s, blk_s] + q // P
    pp = q % P
    flat = cc * P + pp  # flat slot id in [0, C*P)

    slot_idx = np.zeros((NCORES, C * P), np.int16)
    slot_dstl = np.zeros((NCORES, C * P), np.float32)
    slot_norm = np.zeros((NCORES, C * P), np.float32)
    slot_idx[core_s, flat] = ridx[order]
    slot_dstl[core_s, flat] = dl[order]
    slot_norm[core_s, flat] = norm_e[order]

    # Per-(core, call) runtime gather count: whole trailing padding chunks are
    # marked -1 (the gather ucode truncates trailing negatives) and the count
    # register is rounded to the same 128 boundary so the decode-side ring
    # reservation matches the ucode's descriptor count exactly. Partial-chunk
    # padding keeps idx 0 (dummy descriptors cost the same either way).
    real = np.zeros((NCORES, C * P), bool)
    real[core_s, flat] = True
    ncalls_pad = -(-len(calls) // 4) * 4
    nidx_call = np.zeros((NCORES, ncalls_pad), np.int32)
    for ci, (_r, c0, ncall, _g) in enumerate(calls):
        s0, s1 = c0 * P, (c0 + ncall) * P
        seg = real[:, s0:s1]
        rev = seg[:, ::-1]
        has = rev.any(axis=1)
        last = np.where(has, (s1 - s0 - 1) - rev.argmax(axis=1), -1)
        kept = (last // P + 1) * P          # 128-rounded per-core count
        nidx_call[:, ci] = kept
        if V_REG:
            mask = np.arange(s1 - s0)[None, :] >= kept[:, None]
            slot_idx[:, s0:s1][mask] = -1

    # idx16 layout [16, 8*C] wrapped, replicated to 128 partitions
    arr = slot_idx.reshape(NCORES, C, 8, 16)            # [c, cc, pp//16, pp%16]
    idx16s = arr.transpose(0, 3, 1, 2).reshape(NCORES, 16, 8 * C)
    idx16 = np.tile(idx16s, (1, 8, 1))                  # [c, 128, 8C]

    dstl = slot_dstl.reshape(NCORES, C, P).transpose(0, 2, 1).astype(np.float16)
    normv = slot_norm.reshape(NCORES, C, P).transpose(0, 2, 1).astype(np.float16)
    normf = slot_norm.reshape(NCORES, C, P).transpose(0, 2, 1).astype(np.float32)

    d2 = np.zeros((NCORES, NPCP), np.float32)
    for c in range(NCORES):
        d2[c, :NPC] = dinv2[c * NPC:(c + 1) * NPC]
    dinv2o = d2.reshape(NCORES, NBLK, P).transpose(0, 2, 1).copy()

    xT = np.zeros((NCORES, P, NPCP), np.float16)
    xf = np.asarray(x, dtype=np.float32)
    for c in range(NCORES):
        xT[c, :, :NPC] = xf[c * NPC:(c + 1) * NPC].T.astype(np.float16)

    iota = np.broadcast_to(np.arange(P, dtype=np.float16), (P, P)).copy()
    ident = np.eye(P, dtype=np.float32)

    consts = dict(
        W1=np.asarray(W1, np.float32).astype(np.float16),
        W2=np.asarray(W2, np.float32).astype(np.float16),
        Wres03=(0.3 * np.asarray(Wres, np.float32)).astype(np.float16),
        iota=iota, ident=ident,
    )
    b1 = np.asarray(b1, np.float32)
    b2 = np.asarray(b2, np.float32)
    bres03 = 0.3 * np.asarray(bres, np.float32)
    has_b1 = bool(np.any(b1)) or bool(np.any(bres03))
    has_b2 = bool(np.any(b2))
    if has_b1:
        consts["bias1"] = np.broadcast_to(b1, (P, P)).copy()
        consts["bres03"] = np.broadcast_to(bres03, (P, P)).copy()
    if has_b2:
        consts["bias2"] = np.broadcast_to(b2, (P, P)).copy()

    in_maps = []
    for c in range(NCORES):
        m = dict(consts)
        m.update(xT=xT[c], idx16=idx16[c], dstl=dstl[c], normv=normv[c],
                 dinv2o=dinv2o[c], ncnt=nidx_call[c:c + 1], normf=normf[c])
        in_maps.append(m)
    call_sig = tuple((r, c0, ncall, tuple(g)) for r, c0, ncall, g in calls)
    return C, call_sig, has_b1, has_b2, in_maps


def _build(C, calls, has_b1, has_b2):
    import concourse.bacc as bacc
    import concourse.bass as bass
    import concourse.mybir as mybir
    import concourse.tile as tile

    f32 = mybir.dt.float32
    f16 = mybir.dt.float16
    i16 = mybir.dt.int16
    EQ = mybir.AluOpType.is_equal
    MUL = mybir.AluOpType.mult
    ADD = mybir.AluOpType.add
    GELU = mybir.ActivationFunctionType.Gelu
    COPY = mybir.ActivationFunctionType.Copy

    nc = bacc.Bacc(num_swdge_queues=4)

    xT_d = nc.dram_tensor("xT", [P, NPCP], f16, kind="ExternalInput")
    W1_d = nc.dram_tensor("W1", [P, P], f16, kind="ExternalInput")
    W2_d = nc.dram_tensor("W2", [P, P], f16, kind="ExternalInput")
    Wres_d = nc.dram_tensor("Wres03", [P, P], f16, kind="ExternalInput")
    iota_d = nc.dram_tensor("iota", [P, P], f16, kind="ExternalInput")
    ident_d = nc.dram_tensor("ident", [P, P], f32, kind="ExternalInput")
    idx_d = nc.dram_tensor("idx16", [P, 8 * C], i16, kind="ExternalInput")
    dstl_d = nc.dram_tensor("dstl", [P, C], f16, kind="ExternalInput")
    norm_d = nc.dram_tensor("normv", [P, C], f16, kind="ExternalInput")
    dinv2_d = nc.dram_tensor("dinv2o", [P, NBLK], f32, kind="ExternalInput")
    ncalls_pad = -(-len(calls) // 4) * 4
    ncnt_d = nc.dram_tensor("ncnt", [1, ncalls_pad], mybir.dt.int32,
                            kind="ExternalInput")
    normf_d = nc.dram_tensor("normf", [P, C], f32, kind="ExternalInput")
    bias1_d = nc.dram_tensor("bias1", [P, P], f32, kind="ExternalInput") if has_b1 else None
    bres_d = nc.dram_tensor("bres03", [P, P], f32, kind="ExternalInput") if has_b1 else None
    bias2_d = nc.dram_tensor("bias2", [P, P], f32, kind="ExternalInput") if has_b2 else None

    out_d = nc.dram_tensor("out", [NPCP, D], f32, kind="ExternalOutput")

    xw1_own = nc.dram_tensor("xw1_own", [NPCP, D], f16)
    xr03_dr = nc.dram_tensor("xr03", [NPCP, D], f16)
    xw2_own = nc.dram_tensor("xw2_own", [NPCP, D], f16)
    table1 = nc.dram_tensor("table1", [TR, D], f16, addr_space="Shared")
    table2 = nc.dram_tensor("table2", [TR, D], f16, addr_space="Shared")

    rg = [list(range(NCORES))]
    GBUFS = 12
    qctr = [0]

    with tile.TileContext(nc) as tc:
        with (
            tc.tile_pool(name="meta", bufs=1) as mp,
            tc.tile_pool(name="gp", bufs=GBUFS) as gp,
            tc.tile_pool(name="sp", bufs=6) as sp,
            tc.tile_pool(name="wk", bufs=3) as wk,
            tc.tile_pool(name="pz", bufs=3, space="PSUM") as pz,
            tc.tile_pool(name="pa", bufs=2, space="PSUM") as pa,
        ):
            # ---- resident tiles
            w1_t = mp.tile([P, P], f16)
            w2_t = mp.tile([P, P], f16)
            wr_t = mp.tile([P, P], f16)
            iota_t = mp.tile([P, P], f16)
            id_t = mp.tile([P, P], f32)
            idx_t = mp.tile([P, 8 * C], i16)
            dstl_t = mp.tile([P, C], f16)
            norm_t = mp.tile([P, C], f16)
            dinv2_t = mp.tile([P, NBLK], f32)
            zacc = mp.tile([P, NBLK * P], f16)
            ncnt_t = mp.tile([1, ncalls_pad], mybir.dt.int32)
            nc.sync.dma_start(out=ncnt_t[:], in_=ncnt_d[:])
            normf_t = mp.tile([P, C], f32)
            nc.sync.dma_start(out=normf_t[:], in_=normf_d[:])
            nc.sync.dma_start(out=w1_t[:], in_=W1_d[:])
            nc.sync.dma_start(out=w2_t[:], in_=W2_d[:])
            nc.sync.dma_start(out=wr_t[:], in_=Wres_d[:])
            nc.sync.dma_start(out=iota_t[:], in_=iota_d[:])
            nc.sync.dma_start(out=id_t[:], in_=ident_d[:])
            nc.sync.dma_start(out=idx_t[:], in_=idx_d[:])
            nc.sync.dma_start(out=dstl_t[:], in_=dstl_d[:])
            nc.sync.dma_start(out=norm_t[:], in_=norm_d[:])
            nc.sync.dma_start(out=dinv2_t[:], in_=dinv2_d[:])
            if has_b1:
                bias1_t = mp.tile([P, P], f32)
                bres_t = mp.tile([P, P], f32)
                nc.sync.dma_start(out=bias1_t[:], in_=bias1_d[:])
                nc.sync.dma_start(out=bres_t[:], in_=bres_d[:])
            if has_b2:
                bias2_t = mp.tile([P, P], f32)
                nc.sync.dma_start(out=bias2_t[:], in_=bias2_d[:])

            # zero the gather buffers once: truncated gather slots leave SBUF
            # untouched, and the matmul multiplies them by 0 — stale NaNs from
            # uninitialized SBUF would poison 0*NaN. After this, reuses only
            # ever contain previous gathered table rows (finite).
            for _ in range(GBUFS):
                g0 = gp.tile([P, MAXCH, D], f16, tag="g")
                nc.vector.memset(g0[:], 0)

            # ---- phase A: xw1 = x@W1, xr03 = x@(0.3*Wres), shard-local;
            # fire quarter-AllGathers as their blocks complete.
            qend = [QSTART[j] + QB[j] - 1 for j in range(NRANGE)]
            for t in range(NBLK):
                xt = wk.tile([P, P], f16, tag="xt")
                nc.sync.dma_start(out=xt[:], in_=xT_d[:, t * P:(t + 1) * P])
                ps1 = pa.tile([P, P], f32, space="PSUM", tag="ps1")
                ps2 = pa.tile([P, P], f32, space="PSUM", tag="ps2")
                nc.tensor.matmul(ps1[:], xt[:], w1_t[:], start=True, stop=True)
                nc.tensor.matmul(ps2[:], xt[:], wr_t[:], start=True, stop=True)
                c1 = wk.tile([P, P], f16, tag="c1")
                nc.vector.tensor_copy(out=c1[:], in_=ps1[:])
                c2 = wk.tile([P, P], f16, tag="c2")
                if has_b1:
                    nc.vector.tensor_add(out=c2[:], in0=ps2[:], in1=bres_t[:])
                else:
                    nc.scalar.activation(out=c2[:], in_=ps2[:], func=COPY)
                nc.sync.dma_start(out=xw1_own[t * P:(t + 1) * P, :], in_=c1[:])
                nc.sync.dma_start(out=xr03_dr[t * P:(t + 1) * P, :], in_=c2[:])
                if V_AGINLINE:
                    for j in range(NRANGE):
                        if t == qend[j]:
                            r0, r1 = QSTART[j] * P, (QSTART[j] + QB[j]) * P
                            nc.gpsimd.collective_compute(
                                "AllGather", mybir.AluOpType.bypass,
                                replica_groups=rg,
                                ins=[xw1_own[r0:r1, :]],
                                outs=[table1[RBASE[j]:RBASE[j] + RS[j], :]],
                            )
            if not V_AGINLINE:
                for j in range(NRANGE):
                    r0, r1 = QSTART[j] * P, (QSTART[j] + QB[j]) * P
                    nc.gpsimd.collective_compute(
                        "AllGather", mybir.AluOpType.bypass,
                        replica_groups=rg,
                        ins=[xw1_own[r0:r1, :]],
                        outs=[table1[RBASE[j]:RBASE[j] + RS[j], :]],
                    )

            # ---- aggregation pass: range-major gathers into zacc (fp16)
            nregs = [nc.gpsimd.alloc_register(f"nidx{i}") for i in range(4)]

            def agg_pass(table_d, epilogue_cb):
                for ci, (r, c0, ncall, groups) in enumerate(calls):
                    gb = gp.tile([P, MAXCH, D], f16, tag="g")
                    if V_REG and ci % 4 == 0:
                        nc.gpsimd.reg_load(nregs, ncnt_t[0:1, ci:ci + 4])
                    nc.gpsimd.dma_gather(
                        out_ap=gb[:, :ncall, :],
                        in_ap=table_d[RBASE[r]:RBASE[r] + RS[r], :],
                        idxs_ap=idx_t[:, 8 * c0:8 * (c0 + ncall)],
                        num_idxs=P * ncall,
                        num_idxs_reg=nregs[ci % 4] if V_REG else P * ncall,
                        elem_size=D,
                        queue_num=qctr[0] % 4,
                    )
                    qctr[0] += 1
                    for (b, g0, kk) in groups:
                        sw = sp.tile([P, MAXCH, P], f16, tag="sw")
                        if V_BATCH:
                            io_b = iota_t[:].unsqueeze(1).broadcast_to([P, kk, P])
                            dst_b = dstl_t[:, g0:g0 + kk].unsqueeze(2).broadcast_to([P, kk, P])
                            nc.vector.tensor_tensor(out=sw[:, :kk, :], in0=io_b, in1=dst_b, op=EQ)
                            if qctr[0] % 2 == 0:
                                nrm_b = norm_t[:, g0:g0 + kk].unsqueeze(2).broadcast_to([P, kk, P])
                                nc.vector.tensor_tensor(out=sw[:, :kk, :], in0=sw[:, :kk, :], in1=nrm_b, op=MUL)
                            else:
                                for j in range(kk):
                                    nc.scalar.activation(
                                        out=sw[:, j, :], in_=sw[:, j, :], func=COPY,
                                        scale=normf_t[:, g0 + j:g0 + j + 1])
                        else:
                            for j in range(kk):
                                nc.vector.tensor_scalar(
                                    out=sw[:, j, :], in0=iota_t[:],
                                    scalar1=dstl_t[:, g0 + j:g0 + j + 1],
                                    scalar2=norm_t[:, g0 + j:g0 + j + 1],
                                    op0=EQ, op1=MUL,
                                )
                        zp = pz.tile([P, P], f32, space="PSUM", tag="z")
                        for j in range(kk):
                            nc.tensor.matmul(
                                zp[:], sw[:, j, :], gb[:, (g0 - c0) + j, :],
                                start=(j == 0), stop=(j == kk - 1),
                            )
                        zslice = zacc[:, b * P:(b + 1) * P]
                        if r == 0:
                            nc.scalar.activation(out=zslice, in_=zp[:], func=COPY)
                        else:
                            nc.vector.tensor_tensor(out=zslice, in0=zslice, in1=zp[:], op=ADD)
                        if r == NRANGE - 1:
                            epilogue_cb(b)

            if V_STAGE == "A":
                for b in range(NBLK):
                    og = wk.tile([P, P], f32, tag="ge")
                    nc.vector.tensor_copy(out=og[:], in_=zacc[:, b * P:(b + 1) * P])
                    nc.sync.dma_start(out=out_d[b * P:(b + 1) * P, :], in_=og[:])
            def epi1(b):
                ob = wk.tile([P, P], f16, tag="ob")
                nc.sync.dma_start(out=ob[:], in_=xw1_own[b * P:(b + 1) * P, :])
                e2 = wk.tile([P, P], f32, tag="e2")
                nc.vector.scalar_tensor_tensor(
                    out=e2[:], in0=ob[:], scalar=dinv2_t[:, b:b + 1],
                    in1=zacc[:, b * P:(b + 1) * P], op0=MUL, op1=ADD,
                )
                if has_b1:
                    nc.vector.tensor_add(out=e2[:], in0=e2[:], in1=bias1_t[:])
                ge = wk.tile([P, P], f32, tag="ge")
                nc.scalar.activation(out=ge[:], in_=e2[:], func=GELU)
                xr = wk.tile([P, P], f16, tag="xr")
                nc.sync.dma_start(out=xr[:], in_=xr03_dr[b * P:(b + 1) * P, :])
                hb = wk.tile([P, P], f32, tag="hb")
                nc.vector.tensor_tensor(out=hb[:], in0=ge[:], in1=xr[:], op=ADD)
                pt = pa.tile([P, P], f32, space="PSUM", tag="ps2")
                nc.tensor.transpose(out=pt[:], in_=hb[:], identity=id_t[:])
                hc = wk.tile([P, P], f16, tag="hc")
                nc.scalar.activation(out=hc[:], in_=pt[:], func=COPY)
                psC = pa.tile([P, P], f32, space="PSUM", tag="ps1")
                nc.tensor.matmul(psC[:], hc[:], w2_t[:], start=True, stop=True)
                cC = wk.tile([P, P], f16, tag="cC")
                nc.scalar.activation(out=cC[:], in_=psC[:], func=COPY)
                nc.sync.dma_start(out=xw2_own[b * P:(b + 1) * P, :], in_=cC[:])
                if V_AGINLINE:
                    for j in range(NRANGE):
                        if b == qend[j]:
                            r0, r1 = QSTART[j] * P, (QSTART[j] + QB[j]) * P
                            nc.gpsimd.collective_compute(
                                "AllGather", mybir.AluOpType.bypass,
                                replica_groups=rg,
                                ins=[xw2_own[r0:r1, :]],
                                outs=[table2[RBASE[j]:RBASE[j] + RS[j], :]],
                            )

            def epi2(b):
                ob = wk.tile([P, P], f16, tag="ob")
                nc.sync.dma_start(out=ob[:], in_=xw2_own[b * P:(b + 1) * P, :])
                e2 = wk.tile([P, P], f32, tag="e2")
                nc.vector.scalar_tensor_tensor(
                    out=e2[:], in0=ob[:], scalar=dinv2_t[:, b:b + 1],
                    in1=zacc[:, b * P:(b + 1) * P], op0=MUL, op1=ADD,
                )
                if has_b2:
                    nc.vector.tensor_add(out=e2[:], in0=e2[:], in1=bias2_t[:])
                og = wk.tile([P, P], f32, tag="ge")
                nc.scalar.activation(out=og[:], in_=e2[:], func=GELU)
                nc.sync.dma_start(out=out_d[b * P:(b + 1) * P, :], in_=og[:])

            if V_STAGE != "A":
                agg_pass(table1, epi1)
                if not V_AGINLINE:
                    for j in range(NRANGE):
                        r0, r1 = QSTART[j] * P, (QSTART[j] + QB[j]) * P
                        nc.gpsimd.collective_compute(
                            "AllGather", mybir.AluOpType.bypass,
                            replica_groups=rg,
                            ins=[xw2_own[r0:r1, :]],
                            outs=[table2[RBASE[j]:RBASE[j] + RS[j], :]],
                        )
                agg_pass(table2, epi2)

            if V_STAGE == "A1":
                for b in range(NBLK):
                    og = wk.tile([P, P], f32, tag="ge")
                    nc.vector.tensor_copy(out=og[:], in_=zacc[:, b * P:(b + 1) * P])
                    nc.sync.dma_start(out=out_d[b * P:(b + 1) * P, :], in_=og[:])

    nc.compile()
    return nc


def _get_compiled(C, calls, has_b1, has_b2):
    key = (C, calls, has_b1, has_b2, V_AGINLINE, V_REG, V_BATCH, V_STAGE)
    if key not in _CACHE:
        _CACHE[key] = _build(C, calls, has_b1, has_b2)
    return _CACHE[key]


def kernel(x, edge_index, B, N, causal_edge_index, edge_weight,
           causal_edge_weight, W1, b1, W2, b2, Wres, bres):
    assert int(B) * int(N) == NCORES * NPC
    from concourse.bass_utils import run_bass_kernel_spmd

    C, calls, has_b1, has_b2, in_maps = _preprocess(
        x, edge_index, edge_weight, W1, b1, W2, b2, Wres, bres)
    nc = _get_compiled(C, calls, has_b1, has_b2)
    res = run_bass_kernel_spmd(nc, in_maps, list(range(NCORES)))
    out = np.concatenate(
        [res.results[c]["out"][:NPC] for c in range(NCORES)], axis=0)
    return out.astype(np.float32)


# exposed for test.py so it can reuse preprocessing + run with tracing
def _run_traced(x, edge_index, edge_weight, W1, b1, W2, b2, Wres, bres,
                **trace_kwargs):
    from concourse.bass_utils import run_bass_kernel_spmd
    C, calls, has_b1, has_b2, in_maps = _preprocess(
        x, edge_index, edge_weight, W1, b1, W2, b2, Wres, bres)
    nc = _get_compiled(C, calls, has_b1, has_b2)
    res = run_bass_kernel_spmd(nc, in_maps, list(range(NCORES)),
                               **trace_kwargs)
    out = np.concatenate(
        [res.results[c]["out"][:NPC] for c in range(NCORES)], axis=0)
    return out.astype(np.float32), res
